# revision 1
# baseline (speedup 1.0000x reference)
"""Trainium2 Bass kernel for LocalDenseSynthesizerAttention (band C=63, H=4 heads).

Sharding: 8192 tokens (B=2 x T=4096 flattened) split contiguously across 8
cores (1024 tokens each).  Each core runs an identical program on its own
slice; batch-edge band masking and value halo padding are handled host-side
via per-core input data (masks / zero-padded valueT), so the program is
uniform SPMD.

Layouts on device are feature-major ("transposed"): activations live as
(feat, token) so every projection is a plain lhsT.T @ rhs matmul.  The band
scatter uses a diagonal DMA access pattern (flat element step = row_pitch+1)
to turn compact scores (token, offset) into a banded matrix, which is then
PE-transposed into (window_row, token) orientation for the band matmuls.
"""

import numpy as np
import ml_dtypes

import concourse.bass as bass
import concourse.bacc as bacc
import concourse.mybir as mybir
import concourse.tile as tile
from concourse.ap import AP
from concourse import bass_utils

BF16 = mybir.dt.bfloat16
FP32 = mybir.dt.float32
NP_BF16 = ml_dtypes.bfloat16

B, T, NF = 2, 4096, 256
H, C, DK = 4, 63, 64
HALF = (C - 1) // 2  # 31
N_CORES = 8
TPC = (B * T) // N_CORES  # 1024 tokens per core
N_TILES = TPC // 128  # 8
N_SUPER = TPC // 256  # 4
VPAD = 1152  # parked value rows: tokens [-31, 1121) relative to core start
SW = 256  # per-head width of the scatter buffer (window rows padded 190->256)


def build_program(reps: int = 1):
    import contextlib

    nc = bacc.Bacc(
        "TRN2",
        target_bir_lowering=False,
        debug=False,
        enable_asserts=False,
        num_devices=N_CORES,
    )

    # DRAM I/O (per-core data, same names on every core)
    # wpack = [w1T | w2T | w3T | woT | ident(+zero pad)] along the free dim
    qT_d = nc.dram_tensor("qT", [NF, TPC], BF16, kind="ExternalInput").ap()
    vT_d = nc.dram_tensor("vT", [NF, VPAD], BF16, kind="ExternalInput").ap()
    wpack_d = nc.dram_tensor("wpack", [NF, 1148], BF16, kind="ExternalInput").ap()
    maskp_d = nc.dram_tensor("maskp", [128, 2 * H * C], FP32, kind="ExternalInput").ap()
    outT_d = nc.dram_tensor("outT", [NF, TPC], BF16, kind="ExternalOutput").ap()

    with tile.TileContext(nc) as tc:
        with (
            tc.tile_pool(name="inp", bufs=1) as inp,
            tc.tile_pool(name="work", bufs=8) as work,
            tc.tile_pool(name="dram", bufs=1, space="DRAM") as dram,
            tc.tile_pool(name="big_ps", bufs=1, space="PSUM") as big_ps,
            tc.tile_pool(name="sc_ps", bufs=2, space="PSUM") as sc_ps,
            tc.tile_pool(name="tr_ps", bufs=3, space="PSUM") as tr_ps,
            tc.tile_pool(name="x_ps", bufs=2, space="PSUM") as x_ps,
        ):
            # ---- persistent SBUF tensors --------------------------------
            qt_in = inp.tile([128, 2, TPC], BF16, tag="qt_in")
            vt_in = inp.tile([128, 2, VPAD], BF16, tag="vt_in")
            wall = inp.tile([128, 2, 1148], BF16, tag="wall")
            maskp = inp.tile([128, 2 * H * C], FP32, tag="maskp")
            w1t = wall[:, :, 0:256]
            w2t = wall[:, :, 256:508]
            w3t = wall[:, :, 508:764]
            wot = wall[:, :, 764:1020]
            ident = wall[:, 0, 1020:1148]
            mask0 = maskp[:, 0 : H * C]
            mask7 = maskp[:, H * C : 2 * H * C]
            qtr = inp.tile([128, 2, TPC], BF16, tag="qtr")
            vpark = inp.tile([128, 9, NF], BF16, tag="vpark")
            xt = inp.tile([128, 2, TPC], BF16, tag="xt")
            outsb = inp.tile([128, 2, TPC], BF16, tag="outsb")
            # scatter buffers (ping-pong) and super-tile S^T buffers
            sall = [
                inp.tile([128, H * SW], BF16, tag=f"sall{i}", name=f"sall{i}")
                for i in range(8)
            ]
            # DRAM staging for the diagonal scatter (SBUF APs cannot mix
            # partition and element steps; DRAM is flat so they can)
            stage = [
                dram.tile([128, H * SW], BF16, tag=f"stage{i}", name=f"stage{i}")
                for i in range(8)
            ]
            # S^T per tile: (128 window-rows, 4 heads x 2 chunks x 128 tokens)
            stal = [
                inp.tile([128, H * SW], BF16, tag=f"stal{i}", name=f"stal{i}")
                for i in range(8)
            ]

            loop_ctx = tc.For_i(0, reps, 1, hint_engines=(mybir.EngineType.PE,)) if reps > 1 else contextlib.nullcontext()
            with loop_ctx:
                # ---- input DMAs (weights first) ----------------------------
                nc.sync.dma_start(wall[:], wpack_d.rearrange("(c p) t -> p c t", p=128))
                nc.sync.dma_start(maskp[:], maskp_d)
                qT_r = qT_d.rearrange("(c p) t -> p c t", p=128)
                vT_r = vT_d.rearrange("(c p) t -> p c t", p=128)
                nc.sync.dma_start(qt_in[:, :, 0:512], qT_r[:, :, 0:512])
                nc.sync.dma_start(qt_in[:, :, 512:TPC], qT_r[:, :, 512:TPC])
                nc.sync.dma_start(vt_in[:, :, 0:576], vT_r[:, :, 0:576])
                nc.sync.dma_start(vt_in[:, :, 576:VPAD], vT_r[:, :, 576:VPAD])

                # pre-zero the DRAM stages once; band regions are rewritten every
                # tile, everything else stays structurally zero.
                nc.gpsimd.memset(sall[0][:], 0.0)
                for stg in stage:
                    nc.sync.dma_start(stg[:], sall[0][:])

                # ---- stage 1: qTr = relu(w1 @ queryT) ----------------------
                for m in range(2):  # mega-tiles of 512 tokens
                    for mc in range(2):  # output feature chunk
                        ps = big_ps.tile([128, 512], FP32, tag="big")
                        for kc in range(2):
                            nc.tensor.matmul(
                                ps[:],
                                w1t[:, kc, mc * 128 : (mc + 1) * 128],
                                qt_in[:, kc, m * 512 : (m + 1) * 512],
                                start=(kc == 0),
                                stop=(kc == 1),
                            )
                        nc.scalar.activation(
                            qtr[:, mc, m * 512 : (m + 1) * 512],
                            ps[:],
                            mybir.ActivationFunctionType.Relu,
                        )

                # ---- stage 2: V = value @ w3.T parked at -31 offset --------
                for vp in range(5):  # pairs of V tiles share one PSUM bank
                    nv = 2 if vp < 4 else 1
                    ps = big_ps.tile([128, 512], FP32, tag="big")
                    for j in range(nv):
                        vt = 2 * vp + j
                        for kc in range(2):
                            nc.tensor.matmul(
                                ps[:, j * 256 : (j + 1) * 256],
                                vt_in[:, kc, vt * 128 : (vt + 1) * 128],
                                w3t[:, kc, :],
                                start=(kc == 0),
                                stop=(kc == 1),
                            )
                    if vp % 2 == 0:
                        nc.vector.tensor_copy(
                            vpark[:, 2 * vp : 2 * vp + nv, :],
                            ps[:, 0 : nv * 256].rearrange("p (a b) -> p a b", a=nv),
                        )
                    else:
                        nc.scalar.activation(
                            vpark[:, 2 * vp : 2 * vp + nv, :],
                            ps[:, 0 : nv * 256].rearrange("p (a b) -> p a b", a=nv),
                            mybir.ActivationFunctionType.Copy,
                        )

                # ---- pass A per tile: scores -> exp -> norm -> scatter -----
                for t in range(N_TILES):
                    # scores for tile t -> PSUM
                    sc = sc_ps.tile([128, H * C], FP32, tag="sc")
                    for kc in range(2):
                        nc.tensor.matmul(
                            sc[:],
                            qtr[:, kc, t * 128 : (t + 1) * 128],
                            w2t[:, kc, :],
                            start=(kc == 0),
                            stop=(kc == 1),
                        )
                    if t == 0:
                        nc.vector.tensor_add(sc[:], sc[:], mask0)
                    if t == N_TILES - 1:
                        nc.vector.tensor_add(sc[:], sc[:], mask7)
                    # exp, per-head denominators, normalize
                    expp = work.tile([128, H * C], BF16, tag="expp")
                    nc.scalar.activation(
                        expp[:], sc[:], mybir.ActivationFunctionType.Exp
                    )
                    den = work.tile([128, H], FP32, tag="den")
                    nc.vector.tensor_reduce(
                        den[:],
                        expp[:].rearrange("p (h c) -> p h c", h=H),
                        axis=mybir.AxisListType.X,
                        op=mybir.AluOpType.add,
                    )
                    rden = work.tile([128, H], FP32, tag="rden")
                    nc.vector.reciprocal(rden[:], den[:])
                    pn = work.tile([128, H * C], BF16, tag="pn")
                    rb = AP(rden[:].tensor, rden[:].offset, [[H, 128], [1, H], [0, C]])
                    nc.gpsimd.tensor_mul(
                        pn[:].rearrange("p (h c) -> p h c", h=H),
                        expp[:].rearrange("p (h c) -> p h c", h=H),
                        rb,
                    )
                    # diagonal scatter via DRAM stage:
                    #   stage[i*(W+1) + h*SW + k] = pn[i, h*C + k]  (diag write)
                    #   S = stage (rectangular read; zeros persist off-band)
                    stg = stage[t]
                    diag_dst = AP(
                        stg[:].tensor,
                        stg[:].offset,
                        [[H * SW + 1, 128], [SW, H], [1, C]],
                    )
                    nc.sync.dma_start(diag_dst, pn[:].rearrange("p (h c) -> p h c", h=H))
                    nc.sync.dma_start(sall[t][:], stg[:])

                # ---- pass B per tile: transpose -> band matmul -> out ------
                for t in range(N_TILES):
                    st = sall[t]
                    # 8 transposes into one PSUM bank, then a single copy out
                    trp = tr_ps.tile([128, H * SW], BF16, tag="trp")
                    for h in range(H):
                        for c in range(2):
                            o = h * SW + c * 128
                            nc.tensor.transpose(
                                trp[:, o : o + 128], st[:, o : o + 128], ident
                            )
                    sta = stal[t]
                    if t % 2 == 0:
                        nc.vector.tensor_copy(sta[:], trp[:])
                    else:
                        nc.scalar.activation(
                            sta[:], trp[:], mybir.ActivationFunctionType.Copy
                        )
                    # band matmuls: xT_h = V_ext^T @ S^T  (window chunks are
                    # park-tile aligned thanks to the -31 park offset)
                    xps = x_ps.tile([128, 256], FP32, tag="xv")
                    for h in range(H):
                        out_sl = xps[64 * (h % 2) : 64 * (h % 2) + 64,
                                     128 * (h // 2) : 128 * (h // 2) + 128]
                        nc.tensor.matmul(
                            out_sl,
                            vpark[0:128, t, h * DK : (h + 1) * DK],
                            sta[0:128, h * SW : h * SW + 128],
                            start=True,
                            stop=False,
                        )
                        nc.tensor.matmul(
                            out_sl,
                            vpark[0:62, t + 1, h * DK : (h + 1) * DK],
                            sta[0:62, h * SW + 128 : h * SW + 256],
                            start=False,
                            stop=True,
                        )
                    # one copy per tile: (h0,h1 | h2,h3) -> xt feature chunks
                    xdst = AP(
                        xt[:].tensor,
                        xt[:].offset + t * 128,
                        [[2 * TPC, 128], [TPC, 2], [1, 128]],
                    )
                    nc.vector.tensor_copy(
                        xdst, xps[:].rearrange("p (a b) -> p a b", a=2)
                    )

                    # ---- stage 5 interleaved: out-proj per 512-token mega --
                    if t % 4 == 3:
                        m = t // 4
                        outT_r = outT_d.rearrange("(c p) t -> p c t", p=128)
                        for mc in range(2):
                            ps = big_ps.tile([128, 512], FP32, tag="big")
                            for kc in range(2):
                                nc.tensor.matmul(
                                    ps[:],
                                    wot[:, kc, mc * 128 : (mc + 1) * 128],
                                    xt[:, kc, m * 512 : (m + 1) * 512],
                                    start=(kc == 0),
                                    stop=(kc == 1),
                                )
                            if mc == 0:
                                nc.vector.tensor_copy(
                                    outsb[:, mc, m * 512 : (m + 1) * 512], ps[:]
                                )
                            else:
                                nc.scalar.activation(
                                    outsb[:, mc, m * 512 : (m + 1) * 512],
                                    ps[:],
                                    mybir.ActivationFunctionType.Copy,
                                )
                            nc.sync.dma_start(
                                outT_r[:, mc, m * 512 : (m + 1) * 512],
                                outsb[:, mc, m * 512 : (m + 1) * 512],
                            )

    nc.compile()
    return nc


def make_inputs(query, value, w1, w2, w3, w_out):
    """Host-side shard/transpose/cast. Returns per-core in_maps."""
    fq = np.asarray(query, np.float32).reshape(B * T, NF)
    fv = np.asarray(value, np.float32).reshape(B * T, NF)
    wpack = np.zeros((NF, 1148), np.float32)
    wpack[:, 0:256] = np.asarray(w1, np.float32).T
    wpack[:, 256:508] = np.asarray(w2, np.float32).T
    wpack[:, 508:764] = np.asarray(w3, np.float32).T
    wpack[:, 764:1020] = np.asarray(w_out, np.float32).T
    wpack[0:128, 1020:1148] = np.eye(128, dtype=np.float32)
    wpack = wpack.astype(NP_BF16)

    in_maps = []
    for c in range(N_CORES):
        t0 = c * TPC
        b = (c * TPC) // T
        b0, b1 = b * T, (b + 1) * T
        qT = np.ascontiguousarray(fq[t0 : t0 + TPC].T).astype(NP_BF16)
        # parked value rows: global tokens [t0-31, t0-31+VPAD), zero outside batch
        vrows = np.zeros((VPAD, NF), np.float32)
        lo = t0 - HALF
        s0, s1 = max(lo, b0), min(lo + VPAD, b1)
        vrows[s0 - lo : s1 - lo] = fv[s0:s1]
        vT = np.ascontiguousarray(vrows.T).astype(NP_BF16)
        # additive band masks for first/last tile (batch edges only)
        maskp = np.zeros((128, 2 * H * C), np.float32)
        k = np.arange(C)
        for i in range(128):
            g = t0 + i
            bad = (g + k - HALF < b0) | (g + k - HALF >= b1)
            maskp[i, : H * C] = np.tile(np.where(bad, -30000.0, 0.0), H)
            g = t0 + (N_TILES - 1) * 128 + i
            bad = (g + k - HALF < b0) | (g + k - HALF >= b1)
            maskp[i, H * C :] = np.tile(np.where(bad, -30000.0, 0.0), H)
        in_maps.append({"qT": qT, "vT": vT, "wpack": wpack, "maskp": maskp})
    return in_maps


_NC_CACHE = None


def kernel(query, key, value, mask, w1, w2, w3, w_out):
    global _NC_CACHE
    if _NC_CACHE is None:
        _NC_CACHE = build_program()
    nc = _NC_CACHE
    in_maps = make_inputs(query, value, w1, w2, w3, w_out)
    res = bass_utils.run_bass_kernel_spmd(nc, in_maps, core_ids=list(range(N_CORES)))
    outs = []
    for c in range(N_CORES):
        outT = res.results[c]["outT"]  # (256, 1024) fp32
        outs.append(np.ascontiguousarray(outT.T))
    full = np.concatenate(outs, axis=0)  # (8192, 256)
    return full.reshape(B, T, NF).astype(np.float32)

